# revision 8
# baseline (speedup 1.0000x reference)
"""Multi-head attention (16 heads, B=2, S=2048, D=1024) on 8 Trainium2
NeuronCores, tensor-parallel over heads (2 heads per core).

Contract: kernel(**inputs) takes the full unsharded fp32 inputs (as in the
reference nn.Module) and returns (output, attn) as full fp32 arrays.

Per-core program (identical code on all 8 cores; only input DATA differs):
  - inputs arrive pre-transposed/pre-cast on the host: x.T as [D, B*S] fp16,
    per-core weight slices in matmul-ready layouts.
  - Q/K/V projections for the core's 2 heads -> qT/kT [128, B*S] in SBUF
    (feature on partitions: head0 = partitions 0-63, head1 = 64-127).
    V is PE-transposed to natural [seq, feat] layout with an appended
    ones-column per head (fused row-sum trick).
  - Per (batch, local head): scores^T tiles [key=128, query=512] = K_h^T
    x Q_h strips on PE; exp via ACT (scale=1/sqrt(dk)) into fp16 e-bands;
    context^T = [V_h | 1]^T @ E accumulated in PSUM - row dk holds the
    softmax denominators for free.
  - Denominators -> reciprocal -> gpsimd partition-broadcast; DVE
    normalizes the e-bands in place (-> attn output, transposed layout)
    and the context strips.
  - Output projection partial = Wo_slice^T @ context^T -> [D, B*S] fp32
    per-core partial (bias bo fed only to core 0).
Host gathers: attn slabs are transposed back per head; partials are summed.
No cross-core collectives are needed.
"""

import numpy as np

import concourse.bacc as bacc
import concourse.mybir as mybir
import concourse.tile as tile
from concourse import bass_utils

# Problem dims (hardcoded per the harness contract)
B, S, D, H = 2, 2048, 1024, 16
N_CORES = 8

F16 = mybir.dt.float16
F32 = mybir.dt.float32
AF = mybir.ActivationFunctionType


def build_program(b=B, s=S, d=D, h=H, n_cores=N_CORES, e_bufs=20):
    """Build the (SPMD-identical) Bass program. Returns (nc, meta)."""
    dk = d // h                # head dim (64)
    hl = h // n_cores          # heads per core (2)
    f = hl * dk                # local projected features (128)
    seq = b * s                # total rows (4096)
    kcn = d // 128             # contraction chunks for projections (8)
    PS = 512                   # strip width (PSUM bank, fp32)
    nsp = seq // PS            # projection strips (8)
    ni = s // PS               # query strips per (b,h) (4)
    nj = s // 128              # key bands per (b,h) (16)
    nst = seq // 128           # V seq tiles (32)
    net = d // 128             # output-feature tiles (8)
    dk1 = dk + 1               # v cols + ones col
    assert f == 128, "per-core feature count must be 128"

    nc = bacc.Bacc("TRN2", target_bir_lowering=False, debug=False,
                   num_devices=n_cores)

    # ---- DRAM tensors (per-core views) ----
    xq = nc.dram_tensor("xq", [d, seq], F16, kind="ExternalInput").ap()
    xk = nc.dram_tensor("xk", [d, seq], F16, kind="ExternalInput").ap()
    xv = nc.dram_tensor("xv", [d, seq], F16, kind="ExternalInput").ap()
    wq = nc.dram_tensor("wq", [128, kcn * f], F16, kind="ExternalInput").ap()
    wk = nc.dram_tensor("wk", [128, kcn * f], F16, kind="ExternalInput").ap()
    wv = nc.dram_tensor("wv", [128, kcn * f], F16, kind="ExternalInput").ap()
    wo = nc.dram_tensor("wo", [f, d], F16, kind="ExternalInput").ap()
    bq = nc.dram_tensor("bq", [f, 1], F32, kind="ExternalInput").ap()
    bk = nc.dram_tensor("bk", [f, 1], F32, kind="ExternalInput").ap()
    bv = nc.dram_tensor("bv", [f, 1], F32, kind="ExternalInput").ap()
    bo = nc.dram_tensor("bo", [128, net], F32, kind="ExternalInput").ap()
    ident = nc.dram_tensor("ident", [128, 128], F16, kind="ExternalInput").ap()

    attn_o = nc.dram_tensor("attn_o", [b, hl, s, s], F16,
                            kind="ExternalOutput").ap()
    out_o = nc.dram_tensor("out_o", [d, seq], F32, kind="ExternalOutput").ap()

    scale = 1.0 / float(np.sqrt(dk))

    with tile.TileContext(nc) as tc:
        with tc.tile_pool(name="const", bufs=1) as cpool, \
             tc.tile_pool(name="qk", bufs=1) as qkpool, \
             tc.tile_pool(name="vsb", bufs=nst) as vpool, \
             tc.tile_pool(name="ctxp", bufs=2) as ctxpool, \
             tc.tile_pool(name="invp", bufs=2) as invpool, \
             tc.tile_pool(name="obufp", bufs=2) as opool:

            # ---- constants ----
            wq_sb = cpool.tile([128, kcn * f], F16, tag="wq")
            wk_sb = cpool.tile([128, kcn * f], F16, tag="wk")
            wv_sb = cpool.tile([128, kcn * f], F16, tag="wv")
            wo_sb = cpool.tile([f, d], F16, tag="wo")
            bq_sb = cpool.tile([f, 1], F32, tag="bq")
            bk_sb = cpool.tile([f, 1], F32, tag="bk")
            bv_sb = cpool.tile([f, 1], F32, tag="bv")
            bo_sb = cpool.tile([128, net], F32, tag="bo")
            id_sb = cpool.tile([128, 128], F16, tag="id")
            for sb, dr in ((wq_sb, wq), (wk_sb, wk), (wv_sb, wv),
                           (wo_sb, wo), (bq_sb, bq), (bk_sb, bk),
                           (bv_sb, bv), (bo_sb, bo), (id_sb, ident)):
                nc.sync.dma_start(sb[:], dr[:])

            qt = qkpool.tile([f, seq], F16, tag="qt")
            kt = qkpool.tile([f, seq], F16, tag="kt")
            vsb = [vpool.tile([128, hl * dk1], F16, tag="v", name="v")
                   for _ in range(nst)]

            # ---- projections (feature-on-partition layout) ----
            with tc.tile_pool(name="vt", bufs=1) as vtpool:
                vt = vtpool.tile([f, seq], F16, tag="vt")
                with tc.tile_pool(name="xband", bufs=3) as xpool, \
                     tc.tile_pool(name="pproj", bufs=nsp,
                                  space="PSUM") as ppool:
                    for x_dr, w_sb, b_sb, dest in (
                            (xq, wq_sb, bq_sb, qt),
                            (xk, wk_sb, bk_sb, kt),
                            (xv, wv_sb, bv_sb, vt)):
                        psums = [ppool.tile([128, PS], F32, tag="pp", name="pp")
                                 for _ in range(nsp)]
                        for kc in range(kcn):
                            xb = xpool.tile([128, seq], F16, tag="xb")
                            nc.sync.dma_start(
                                xb[:], x_dr[kc * 128:(kc + 1) * 128, :])
                            for si in range(nsp):
                                nc.tensor.matmul(
                                    psums[si][:],
                                    lhsT=w_sb[:, kc * f:(kc + 1) * f],
                                    rhs=xb[:, si * PS:(si + 1) * PS],
                                    start=(kc == 0), stop=(kc == kcn - 1))
                        for si in range(nsp):
                            nc.vector.tensor_scalar_add(
                                dest[:, si * PS:(si + 1) * PS],
                                psums[si][:], b_sb[:])

                # ---- V -> natural [seq, feat] tiles with ones columns ----
                with tc.tile_pool(name="ptr", bufs=4, space="PSUM") as ptpool:
                    for st in range(nst):
                        pt = ptpool.tile([128, 128], F16, tag="pt")
                        nc.tensor.transpose(
                            pt[:], vt[:, st * 128:(st + 1) * 128], id_sb[:])
                        for hh in range(hl):
                            nc.vector.tensor_copy(
                                vsb[st][:, hh * dk1:hh * dk1 + dk],
                                pt[:, hh * dk:(hh + 1) * dk])
                            nc.vector.memset(
                                vsb[st][:, hh * dk1 + dk:hh * dk1 + dk1], 1.0)

            # ---- attention + output projection ----
            with tc.tile_pool(name="eband", bufs=e_bufs) as epool, \
                 tc.tile_pool(name="psc", bufs=4, space="PSUM") as scpool, \
                 tc.tile_pool(name="pctx", bufs=ni, space="PSUM") as cxpool:
                for bb in range(b):
                    ctxb = ctxpool.tile([f, s], F16, tag="cb")
                    for hh in range(hl):
                        q_ap = qt[hh * dk:(hh + 1) * dk, bb * s:(bb + 1) * s]
                        k_ap = kt[hh * dk:(hh + 1) * dk, bb * s:(bb + 1) * s]
                        ebands = [epool.tile([128, s], F16, tag="eb", name="eb")
                                  for _ in range(nj)]
                        cps = [cxpool.tile([dk1, PS], F32, tag="cx", name="cx")
                               for _ in range(ni)]

                        def ctx_mm(jb):
                            vtile = vsb[bb * (s // 128) + jb]
                            for isx in range(ni):
                                nc.tensor.matmul(
                                    cps[isx][:],
                                    lhsT=vtile[:, hh * dk1:(hh + 1) * dk1],
                                    rhs=ebands[jb][:, isx * PS:(isx + 1) * PS],
                                    start=(jb == 0), stop=(jb == nj - 1))

                        # scores^T + exp, software-pipelined one band ahead
                        # of the context matmuls so PE never waits on ACT.
                        for jb in range(nj):
                            for isx in range(ni):
                                ps = scpool.tile([128, PS], F32, tag="sc")
                                nc.tensor.matmul(
                                    ps[:],
                                    lhsT=k_ap[:, jb * 128:(jb + 1) * 128],
                                    rhs=q_ap[:, isx * PS:(isx + 1) * PS],
                                    start=True, stop=True)
                                nc.scalar.activation(
                                    ebands[jb][:, isx * PS:(isx + 1) * PS],
                                    ps[:], AF.Exp, scale=scale)
                            if jb > 0:
                                ctx_mm(jb - 1)
                        ctx_mm(nj - 1)

                        # softmax denominators -> broadcast reciprocal
                        # (reciprocal stays on partition dk; a tiny
                        # SBUF->SBUF DMA shifts it to partition 0 for the
                        # gpsimd broadcast)
                        inv64 = invpool.tile([dk1, s], F16, tag="iv64",
                                             name="iv64")
                        with nc.allow_low_precision(
                                reason="1/denominator in fp16 (~5e-4 rel) "
                                       "matches the fp16 attn output rounding"):
                            for isx in range(ni):
                                nc.vector.reciprocal(
                                    inv64[dk:dk1, isx * PS:(isx + 1) * PS],
                                    cps[isx][dk:dk1, :])
                        inv16 = invpool.tile([1, s], F16, tag="iv16")
                        nc.sync.dma_start(inv16[:], inv64[dk:dk1, :])
                        ivrep = invpool.tile([128, s], F16, tag="ivp")
                        nc.gpsimd.partition_broadcast(ivrep[:], inv16[:])

                        # normalize context strips into ctxb (h1 lands on
                        # partitions 0-63 and is DMA-shifted to 64-127)
                        if hh == 0:
                            for isx in range(ni):
                                nc.vector.tensor_mul(
                                    ctxb[0:dk, isx * PS:(isx + 1) * PS],
                                    cps[isx][0:dk, :],
                                    ivrep[0:dk, isx * PS:(isx + 1) * PS])
                        else:
                            ctmp = ctxpool.tile([dk, s], F16, tag="ct")
                            for isx in range(ni):
                                nc.vector.tensor_mul(
                                    ctmp[:, isx * PS:(isx + 1) * PS],
                                    cps[isx][0:dk, :],
                                    ivrep[0:dk, isx * PS:(isx + 1) * PS])
                            nc.sync.dma_start(ctxb[dk:f, :], ctmp[:])

                        # normalize e-bands in place and store attn^T
                        for jb in range(nj):
                            nc.vector.tensor_mul(
                                ebands[jb][:], ebands[jb][:], ivrep[:])
                            nc.sync.dma_start(
                                attn_o[bb, hh, jb * 128:(jb + 1) * 128, :],
                                ebands[jb][:])

                    # ---- output projection partial for batch bb ----
                    for et in range(net):
                        ob = opool.tile([128, s], F32, tag="ob")
                        for isx in range(ni):
                            ps = scpool.tile([128, PS], F32, tag="sc")
                            nc.tensor.matmul(
                                ps[:],
                                lhsT=wo_sb[:, et * 128:(et + 1) * 128],
                                rhs=ctxb[:, isx * PS:(isx + 1) * PS],
                                start=True, stop=True)
                            nc.vector.tensor_scalar_add(
                                ob[:, isx * PS:(isx + 1) * PS],
                                ps[:], bo_sb[:, et:et + 1])
                        nc.sync.dma_start(
                            out_o[et * 128:(et + 1) * 128,
                                  bb * s:(bb + 1) * s], ob[:])

    nc.compile()
    meta = dict(b=b, s=s, d=d, h=h, n_cores=n_cores, dk=dk, hl=hl, f=f,
                seq=seq, net=net)
    return nc, meta


def shard_inputs(inputs, meta):
    """Full fp32 inputs -> per-core in_maps (host-side layout prep only)."""
    b, s, d, h = meta["b"], meta["s"], meta["d"], meta["h"]
    n_cores, hl, f, net = meta["n_cores"], meta["hl"], meta["f"], meta["net"]
    kcn = d // 128
    seq = b * s

    def xt16(x):
        x16 = np.asarray(x, np.float32).reshape(seq, d).astype(np.float16)
        return np.ascontiguousarray(x16.T)

    XQ, XK, XV = (xt16(inputs[k]) for k in ("query", "key", "value"))
    ident = np.eye(128, dtype=np.float16)

    def wslice(W, fsel):
        # W[fsel].T as SBUF layout [128, kcn*f]
        wt = np.asarray(W, np.float32)[fsel, :].T.astype(np.float16)
        return np.ascontiguousarray(
            wt.reshape(kcn, 128, f).transpose(1, 0, 2).reshape(128, kcn * f))

    Wo = np.asarray(inputs["Wo"], np.float32)
    bo_tiled = np.ascontiguousarray(
        np.asarray(inputs["bo"], np.float32).reshape(net, 128).T)
    in_maps = []
    for c in range(n_cores):
        fsel = slice(c * f, (c + 1) * f)
        in_maps.append({
            "xq": XQ, "xk": XK, "xv": XV,
            "wq": wslice(inputs["Wq"], fsel),
            "wk": wslice(inputs["Wk"], fsel),
            "wv": wslice(inputs["Wv"], fsel),
            "wo": np.ascontiguousarray(
                Wo[:, fsel].T.astype(np.float16)),
            "bq": np.asarray(inputs["bq"], np.float32)[fsel].reshape(f, 1),
            "bk": np.asarray(inputs["bk"], np.float32)[fsel].reshape(f, 1),
            "bv": np.asarray(inputs["bv"], np.float32)[fsel].reshape(f, 1),
            "bo": bo_tiled if c == 0 else np.zeros((128, net), np.float32),
            "ident": ident,
        })
    return in_maps


def assemble_outputs(results, meta):
    b, s, d, h = meta["b"], meta["s"], meta["d"], meta["h"]
    n_cores, hl, seq = meta["n_cores"], meta["hl"], meta["seq"]
    attn = np.empty((b, h, s, s), np.float32)
    out_acc = np.zeros((d, seq), np.float32)
    for c in range(n_cores):
        a = results[c]["attn_o"]  # [b, hl, s(key), s(query)] fp16
        for bb in range(b):
            for hh in range(hl):
                attn[bb, c * hl + hh] = a[bb, hh].T.astype(np.float32)
        out_acc += results[c]["out_o"]
    out = np.ascontiguousarray(out_acc.T).reshape(b, s, d)
    return out, attn


_PROGRAM_CACHE = {}


def _get_program():
    if "nc" not in _PROGRAM_CACHE:
        nc, meta = build_program()
        _PROGRAM_CACHE["nc"] = nc
        _PROGRAM_CACHE["meta"] = meta
    return _PROGRAM_CACHE["nc"], _PROGRAM_CACHE["meta"]


def kernel(**inputs):
    nc, meta = _get_program()
    in_maps = shard_inputs(inputs, meta)
    res = bass_utils.run_bass_kernel_spmd(
        nc, in_maps, core_ids=list(range(meta["n_cores"])))
    return assemble_outputs(res.results, meta)


# revision 11
# speedup vs baseline: 1.1076x; 1.1076x over previous
"""Multi-head attention (16 heads, B=2, S=2048, D=1024) on 8 Trainium2
NeuronCores, tensor-parallel over heads (2 heads per core).

Contract: kernel(**inputs) takes the full unsharded fp32 inputs (as in the
reference nn.Module) and returns (output, attn) as full fp32 arrays.

Per-core program (identical code on all 8 cores; only input DATA differs):
  - inputs arrive pre-transposed/pre-cast on the host: x.T as [D, B*S] fp16,
    per-core weight slices in matmul-ready layouts.
  - Q/K/V projections for the core's 2 heads -> qT/kT [128, B*S] in SBUF
    (feature on partitions: head0 = partitions 0-63, head1 = 64-127).
    V is PE-transposed to natural [seq, feat] layout with an appended
    ones-column per head (fused row-sum trick).
  - Per (batch, local head): scores^T tiles [key=128, query=512] = K_h^T
    x Q_h strips on PE; exp via ACT (scale=1/sqrt(dk)) into fp16 e-bands;
    context^T = [V_h | 1]^T @ E accumulated in PSUM - row dk holds the
    softmax denominators for free.
  - Denominators -> reciprocal -> gpsimd partition-broadcast; DVE
    normalizes the e-bands in place (-> attn output, transposed layout)
    and the context strips.
  - Output projection partial = Wo_slice^T @ context^T -> [D, B*S] fp32
    per-core partial (bias bo fed only to core 0).
Host gathers: attn slabs are transposed back per head; partials are summed.
No cross-core collectives are needed.
"""

import numpy as np

import concourse.bacc as bacc
import concourse.mybir as mybir
import concourse.tile as tile
from concourse import bass_utils

# Problem dims (hardcoded per the harness contract)
B, S, D, H = 2, 2048, 1024, 16
N_CORES = 8

F16 = mybir.dt.float16
F32 = mybir.dt.float32
AF = mybir.ActivationFunctionType


def build_program(b=B, s=S, d=D, h=H, n_cores=N_CORES, e_bufs=22):
    """Build the (SPMD-identical) Bass program. Returns (nc, meta)."""
    dk = d // h                # head dim (64)
    hl = h // n_cores          # heads per core (2)
    f = hl * dk                # local projected features (128)
    seq = b * s                # total rows (4096)
    kcn = d // 128             # contraction chunks for projections (8)
    PS = 512                   # strip width (PSUM bank, fp32)
    nsp = seq // PS            # projection strips (8)
    ni = s // PS               # query strips per (b,h) (4)
    nj = s // 128              # key bands per (b,h) (16)
    nst = seq // 128           # V seq tiles (32)
    net = d // 128             # output-feature tiles (8)
    dk1 = dk + 1               # v cols + ones col
    assert f == 128, "per-core feature count must be 128"

    nc = bacc.Bacc("TRN2", target_bir_lowering=False, debug=False,
                   num_devices=n_cores)

    # ---- DRAM tensors (per-core views) ----
    xq = nc.dram_tensor("xq", [d, seq], F16, kind="ExternalInput").ap()
    xk = nc.dram_tensor("xk", [d, seq], F16, kind="ExternalInput").ap()
    xv = nc.dram_tensor("xv", [d, seq], F16, kind="ExternalInput").ap()
    wq = nc.dram_tensor("wq", [128, kcn * f], F16, kind="ExternalInput").ap()
    wk = nc.dram_tensor("wk", [128, kcn * f], F16, kind="ExternalInput").ap()
    wv = nc.dram_tensor("wv", [128, kcn * f], F16, kind="ExternalInput").ap()
    wo = nc.dram_tensor("wo", [f, d], F16, kind="ExternalInput").ap()
    bq = nc.dram_tensor("bq", [f, 1], F32, kind="ExternalInput").ap()
    bk = nc.dram_tensor("bk", [f, 1], F32, kind="ExternalInput").ap()
    bv = nc.dram_tensor("bv", [f, 1], F32, kind="ExternalInput").ap()
    bo = nc.dram_tensor("bo", [128, net], F32, kind="ExternalInput").ap()
    ident = nc.dram_tensor("ident", [128, 128], F16, kind="ExternalInput").ap()

    attn_o = nc.dram_tensor("attn_o", [b, hl, s, s], F16,
                            kind="ExternalOutput").ap()
    out_o = nc.dram_tensor("out_o", [d, seq], F32, kind="ExternalOutput").ap()

    scale = 1.0 / float(np.sqrt(dk))

    with tile.TileContext(nc) as tc:
        with tc.tile_pool(name="const", bufs=1) as cpool, \
             tc.tile_pool(name="qk", bufs=1) as qkpool, \
             tc.tile_pool(name="vsb", bufs=nst) as vpool, \
             tc.tile_pool(name="ctxp", bufs=2) as ctxpool, \
             tc.tile_pool(name="invp", bufs=2) as invpool, \
             tc.tile_pool(name="obufp", bufs=2) as opool:

            # ---- constants ----
            wq_sb = cpool.tile([128, kcn * f], F16, tag="wq")
            wk_sb = cpool.tile([128, kcn * f], F16, tag="wk")
            wv_sb = cpool.tile([128, kcn * f], F16, tag="wv")
            wo_sb = cpool.tile([f, d], F16, tag="wo")
            bq_sb = cpool.tile([f, 1], F32, tag="bq")
            bk_sb = cpool.tile([f, 1], F32, tag="bk")
            bv_sb = cpool.tile([f, 1], F32, tag="bv")
            bo_sb = cpool.tile([128, net], F32, tag="bo")
            id_sb = cpool.tile([128, 128], F16, tag="id")
            for sb, dr in ((wq_sb, wq), (wk_sb, wk), (wv_sb, wv),
                           (wo_sb, wo), (bq_sb, bq), (bk_sb, bk),
                           (bv_sb, bv), (bo_sb, bo), (id_sb, ident)):
                nc.sync.dma_start(sb[:], dr[:])

            qt = qkpool.tile([f, seq], F16, tag="qt")
            kt = qkpool.tile([f, seq], F16, tag="kt")
            vsb = [vpool.tile([128, hl * dk1], F16, tag="v", name="v")
                   for _ in range(nst)]

            # ---- projections (feature-on-partition layout) ----
            with tc.tile_pool(name="vt", bufs=1) as vtpool:
                vt = vtpool.tile([f, seq], F16, tag="vt")
                with tc.tile_pool(name="xband", bufs=3) as xpool, \
                     tc.tile_pool(name="pproj", bufs=nsp,
                                  space="PSUM") as ppool:
                    for x_dr, w_sb, b_sb, dest in (
                            (xq, wq_sb, bq_sb, qt),
                            (xk, wk_sb, bk_sb, kt),
                            (xv, wv_sb, bv_sb, vt)):
                        psums = [ppool.tile([128, PS], F32, tag="pp", name="pp")
                                 for _ in range(nsp)]
                        for kc in range(kcn):
                            xb = xpool.tile([128, seq], F16, tag="xb")
                            nc.sync.dma_start(
                                xb[:], x_dr[kc * 128:(kc + 1) * 128, :])
                            for si in range(nsp):
                                nc.tensor.matmul(
                                    psums[si][:],
                                    lhsT=w_sb[:, kc * f:(kc + 1) * f],
                                    rhs=xb[:, si * PS:(si + 1) * PS],
                                    start=(kc == 0), stop=(kc == kcn - 1))
                        for si in range(nsp):
                            nc.vector.tensor_scalar_add(
                                dest[:, si * PS:(si + 1) * PS],
                                psums[si][:], b_sb[:])

                # ---- V -> natural [seq, feat] tiles with ones columns ----
                with tc.tile_pool(name="ptr", bufs=4, space="PSUM") as ptpool:
                    for st in range(nst):
                        pt = ptpool.tile([128, 128], F16, tag="pt")
                        nc.tensor.transpose(
                            pt[:], vt[:, st * 128:(st + 1) * 128], id_sb[:])
                        for hh in range(hl):
                            nc.vector.tensor_copy(
                                vsb[st][:, hh * dk1:hh * dk1 + dk],
                                pt[:, hh * dk:(hh + 1) * dk])
                            nc.vector.memset(
                                vsb[st][:, hh * dk1 + dk:hh * dk1 + dk1], 1.0)

            # ---- attention + output projection ----
            with tc.tile_pool(name="eband", bufs=e_bufs) as epool, \
                 tc.tile_pool(name="psc", bufs=2, space="PSUM") as scpool, \
                 tc.tile_pool(name="pctx", bufs=ni, space="PSUM") as cxpool:
                for bb in range(b):
                    ctxb = ctxpool.tile([f, s], F16, tag="cb")
                    for hh in range(hl):
                        q_ap = qt[hh * dk:(hh + 1) * dk, bb * s:(bb + 1) * s]
                        k_ap = kt[hh * dk:(hh + 1) * dk, bb * s:(bb + 1) * s]
                        ebands = [epool.tile([128, s], F16, tag="eb", name="eb")
                                  for _ in range(nj)]
                        cps = [cxpool.tile([dk1, PS], F32, tag="cx", name="cx")
                               for _ in range(ni)]

                        def ctx_mm(jb):
                            vtile = vsb[bb * (s // 128) + jb]
                            for isx in range(ni):
                                nc.tensor.matmul(
                                    cps[isx][:],
                                    lhsT=vtile[:, hh * dk1:(hh + 1) * dk1],
                                    rhs=ebands[jb][:, isx * PS:(isx + 1) * PS],
                                    start=(jb == 0), stop=(jb == nj - 1))

                        # scores^T + exp, software-pipelined one band ahead
                        # of the context matmuls so PE never waits on ACT.
                        # Score strips are paired into [128, 2*PS] PSUM
                        # tiles (2 banks) so each ACT exp op covers 1024
                        # elements, halving per-op overhead.
                        spe = 2 if ni % 2 == 0 else 1   # strips per exp
                        for jb in range(nj):
                            for ih in range(ni // spe):
                                ps = scpool.tile([128, spe * PS], F32,
                                                 tag="sc", name="sc")
                                for half in range(spe):
                                    isx = spe * ih + half
                                    nc.tensor.matmul(
                                        ps[:, half * PS:(half + 1) * PS],
                                        lhsT=k_ap[:, jb * 128:(jb + 1) * 128],
                                        rhs=q_ap[:, isx * PS:(isx + 1) * PS],
                                        start=True, stop=True)
                                nc.scalar.activation(
                                    ebands[jb][:, ih * spe * PS:
                                               (ih + 1) * spe * PS],
                                    ps[:], AF.Exp, scale=scale)
                            if jb > 0:
                                ctx_mm(jb - 1)
                        ctx_mm(nj - 1)

                        # softmax denominators -> broadcast reciprocal
                        # (reciprocal stays on partition dk; a tiny
                        # SBUF->SBUF DMA shifts it to partition 0 for the
                        # gpsimd broadcast)
                        inv64 = invpool.tile([dk1, s], F16, tag="iv64",
                                             name="iv64")
                        with nc.allow_low_precision(
                                reason="1/denominator in fp16 (~5e-4 rel) "
                                       "matches the fp16 attn output rounding"):
                            for isx in range(ni):
                                nc.vector.reciprocal(
                                    inv64[dk:dk1, isx * PS:(isx + 1) * PS],
                                    cps[isx][dk:dk1, :])
                        inv16 = invpool.tile([1, s], F16, tag="iv16")
                        nc.sync.dma_start(inv16[:], inv64[dk:dk1, :])
                        ivrep = invpool.tile([128, s], F16, tag="ivp")
                        nc.gpsimd.partition_broadcast(ivrep[:], inv16[:])

                        # normalize context strips into ctxb (h1 lands on
                        # partitions 0-63 and is DMA-shifted to 64-127)
                        if hh == 0:
                            for isx in range(ni):
                                nc.vector.tensor_mul(
                                    ctxb[0:dk, isx * PS:(isx + 1) * PS],
                                    cps[isx][0:dk, :],
                                    ivrep[0:dk, isx * PS:(isx + 1) * PS])
                        else:
                            ctmp = ctxpool.tile([dk, s], F16, tag="ct")
                            for isx in range(ni):
                                nc.vector.tensor_mul(
                                    ctmp[:, isx * PS:(isx + 1) * PS],
                                    cps[isx][0:dk, :],
                                    ivrep[0:dk, isx * PS:(isx + 1) * PS])
                            nc.sync.dma_start(ctxb[dk:f, :], ctmp[:])

                        # normalize e-bands in place and store attn^T
                        for jb in range(nj):
                            nc.vector.tensor_mul(
                                ebands[jb][:], ebands[jb][:], ivrep[:])
                            nc.sync.dma_start(
                                attn_o[bb, hh, jb * 128:(jb + 1) * 128, :],
                                ebands[jb][:])

                    # ---- output projection partial for batch bb ----
                    for et in range(net):
                        ob = opool.tile([128, s], F32, tag="ob")
                        for isx in range(ni):
                            ps = scpool.tile([128, PS], F32, tag="sc")
                            nc.tensor.matmul(
                                ps[:],
                                lhsT=wo_sb[:, et * 128:(et + 1) * 128],
                                rhs=ctxb[:, isx * PS:(isx + 1) * PS],
                                start=True, stop=True)
                            nc.vector.tensor_scalar_add(
                                ob[:, isx * PS:(isx + 1) * PS],
                                ps[:], bo_sb[:, et:et + 1])
                        nc.sync.dma_start(
                            out_o[et * 128:(et + 1) * 128,
                                  bb * s:(bb + 1) * s], ob[:])

    nc.compile()
    meta = dict(b=b, s=s, d=d, h=h, n_cores=n_cores, dk=dk, hl=hl, f=f,
                seq=seq, net=net)
    return nc, meta


def shard_inputs(inputs, meta):
    """Full fp32 inputs -> per-core in_maps (host-side layout prep only)."""
    b, s, d, h = meta["b"], meta["s"], meta["d"], meta["h"]
    n_cores, hl, f, net = meta["n_cores"], meta["hl"], meta["f"], meta["net"]
    kcn = d // 128
    seq = b * s

    def xt16(x):
        x16 = np.asarray(x, np.float32).reshape(seq, d).astype(np.float16)
        return np.ascontiguousarray(x16.T)

    XQ, XK, XV = (xt16(inputs[k]) for k in ("query", "key", "value"))
    ident = np.eye(128, dtype=np.float16)

    def wslice(W, fsel):
        # W[fsel].T as SBUF layout [128, kcn*f]
        wt = np.asarray(W, np.float32)[fsel, :].T.astype(np.float16)
        return np.ascontiguousarray(
            wt.reshape(kcn, 128, f).transpose(1, 0, 2).reshape(128, kcn * f))

    Wo = np.asarray(inputs["Wo"], np.float32)
    bo_tiled = np.ascontiguousarray(
        np.asarray(inputs["bo"], np.float32).reshape(net, 128).T)
    in_maps = []
    for c in range(n_cores):
        fsel = slice(c * f, (c + 1) * f)
        in_maps.append({
            "xq": XQ, "xk": XK, "xv": XV,
            "wq": wslice(inputs["Wq"], fsel),
            "wk": wslice(inputs["Wk"], fsel),
            "wv": wslice(inputs["Wv"], fsel),
            "wo": np.ascontiguousarray(
                Wo[:, fsel].T.astype(np.float16)),
            "bq": np.asarray(inputs["bq"], np.float32)[fsel].reshape(f, 1),
            "bk": np.asarray(inputs["bk"], np.float32)[fsel].reshape(f, 1),
            "bv": np.asarray(inputs["bv"], np.float32)[fsel].reshape(f, 1),
            "bo": bo_tiled if c == 0 else np.zeros((128, net), np.float32),
            "ident": ident,
        })
    return in_maps


def assemble_outputs(results, meta):
    b, s, d, h = meta["b"], meta["s"], meta["d"], meta["h"]
    n_cores, hl, seq = meta["n_cores"], meta["hl"], meta["seq"]
    attn = np.empty((b, h, s, s), np.float32)
    out_acc = np.zeros((d, seq), np.float32)
    for c in range(n_cores):
        a = results[c]["attn_o"]  # [b, hl, s(key), s(query)] fp16
        for bb in range(b):
            for hh in range(hl):
                attn[bb, c * hl + hh] = a[bb, hh].T.astype(np.float32)
        out_acc += results[c]["out_o"]
    out = np.ascontiguousarray(out_acc.T).reshape(b, s, d)
    return out, attn


_PROGRAM_CACHE = {}


def _get_program():
    if "nc" not in _PROGRAM_CACHE:
        nc, meta = build_program()
        _PROGRAM_CACHE["nc"] = nc
        _PROGRAM_CACHE["meta"] = meta
    return _PROGRAM_CACHE["nc"], _PROGRAM_CACHE["meta"]


def kernel(**inputs):
    nc, meta = _get_program()
    in_maps = shard_inputs(inputs, meta)
    res = bass_utils.run_bass_kernel_spmd(
        nc, in_maps, core_ids=list(range(meta["n_cores"])))
    return assemble_outputs(res.results, meta)


# revision 19
# speedup vs baseline: 1.3490x; 1.2179x over previous
"""Multi-head attention (16 heads, B=2, S=2048, D=1024) on 8 Trainium2
NeuronCores, tensor-parallel over heads (2 heads per core).

Contract: kernel(**inputs) takes the full unsharded fp32 inputs (as in the
reference nn.Module) and returns (output, attn) as full fp32 arrays.

Per-core program (identical code on all 8 cores; only input DATA differs):
  - inputs arrive pre-transposed/pre-cast on the host: x.T as [D, B*S] fp16,
    per-core weight slices in matmul-ready layouts.
  - Q/K/V projections for the core's 2 heads -> qT/kT [128, B*S] in SBUF
    (feature on partitions: head0 = partitions 0-63, head1 = 64-127).
    V is PE-transposed to natural [seq, feat] layout with an appended
    ones-column per head (fused row-sum trick).
  - Per (batch, local head): scores^T tiles [key=128, query=512] = K_h^T
    x Q_h strips on PE; exp via ACT (scale=1/sqrt(dk)) into fp16 e-bands;
    context^T = [V_h | 1]^T @ E accumulated in PSUM - row dk holds the
    softmax denominators for free.
  - Denominators -> reciprocal -> gpsimd partition-broadcast; DVE
    normalizes the e-bands in place (-> attn output, transposed layout)
    and the context strips.
  - Output projection partial = Wo_slice^T @ context^T -> [D, B*S] fp32
    per-core partial (bias bo fed only to core 0).
Host gathers: attn slabs are transposed back per head; partials are summed.
No cross-core collectives are needed.
"""

import numpy as np

import concourse.bacc as bacc
import concourse.mybir as mybir
import concourse.tile as tile
from concourse import bass_utils

# Problem dims (hardcoded per the harness contract)
B, S, D, H = 2, 2048, 1024, 16
N_CORES = 8

F16 = mybir.dt.float16
F32 = mybir.dt.float32
AF = mybir.ActivationFunctionType


def build_program(b=B, s=S, d=D, h=H, n_cores=N_CORES, e_bufs=44):
    """Build the (SPMD-identical) Bass program. Returns (nc, meta)."""
    dk = d // h                # head dim (64)
    hl = h // n_cores          # heads per core (2)
    f = hl * dk                # local projected features (128)
    seq = b * s                # total rows (4096)
    kcn = d // 128             # contraction chunks for projections (8)
    PS = 512                   # strip width (PSUM bank, fp32)
    nsp = seq // PS            # projection strips (8)
    ni = s // PS               # query strips per (b,h) (4)
    nj = s // 128              # key bands per (b,h) (16)
    nst = seq // 128           # V seq tiles (32)
    net = d // 128             # output-feature tiles (8)
    dk1 = dk + 1               # v cols + ones col
    assert f == 128, "per-core feature count must be 128"

    nc = bacc.Bacc("TRN2", target_bir_lowering=False, debug=False,
                   num_devices=n_cores)

    # ---- DRAM tensors (per-core views) ----
    xq = nc.dram_tensor("xq", [d, seq], F16, kind="ExternalInput").ap()
    xk = nc.dram_tensor("xk", [d, seq], F16, kind="ExternalInput").ap()
    xv = nc.dram_tensor("xv", [d, seq], F16, kind="ExternalInput").ap()
    wq = nc.dram_tensor("wq", [128, kcn * f], F16, kind="ExternalInput").ap()
    wk = nc.dram_tensor("wk", [128, kcn * f], F16, kind="ExternalInput").ap()
    wv = nc.dram_tensor("wv", [128, kcn * f], F16, kind="ExternalInput").ap()
    wo = nc.dram_tensor("wo", [f, d], F16, kind="ExternalInput").ap()
    bq = nc.dram_tensor("bq", [f, 1], F32, kind="ExternalInput").ap()
    bk = nc.dram_tensor("bk", [f, 1], F32, kind="ExternalInput").ap()
    bv = nc.dram_tensor("bv", [f, 1], F32, kind="ExternalInput").ap()
    bo = nc.dram_tensor("bo", [128, net], F32, kind="ExternalInput").ap()
    ident = nc.dram_tensor("ident", [128, 128], F16, kind="ExternalInput").ap()

    attn_o = nc.dram_tensor("attn_o", [b, hl, s, s], F16,
                            kind="ExternalOutput").ap()
    out_o = nc.dram_tensor("out_o", [d, seq], F16,
                           kind="ExternalOutput").ap()

    scale = 1.0 / float(np.sqrt(dk))

    with tile.TileContext(nc) as tc:
        with tc.tile_pool(name="const", bufs=1) as cpool, \
             tc.tile_pool(name="qk", bufs=1) as qkpool, \
             tc.tile_pool(name="vsb", bufs=nst) as vpool, \
             tc.tile_pool(name="ctxp", bufs=2) as ctxpool, \
             tc.tile_pool(name="invp", bufs=2) as invpool, \
             tc.tile_pool(name="obufp", bufs=2) as opool:

            # ---- constants ----
            wq_sb = cpool.tile([128, kcn * f], F16, tag="wq")
            wk_sb = cpool.tile([128, kcn * f], F16, tag="wk")
            wv_sb = cpool.tile([128, kcn * f], F16, tag="wv")
            wo_sb = cpool.tile([f, d], F16, tag="wo")
            bq_sb = cpool.tile([f, 1], F32, tag="bq")
            bk_sb = cpool.tile([f, 1], F32, tag="bk")
            bv_sb = cpool.tile([f, 1], F32, tag="bv")
            bo_sb = cpool.tile([128, net], F32, tag="bo")
            id_sb = cpool.tile([128, 128], F16, tag="id")
            for sb, dr in ((wq_sb, wq), (wk_sb, wk), (wv_sb, wv),
                           (wo_sb, wo), (bq_sb, bq), (bk_sb, bk),
                           (bv_sb, bv), (bo_sb, bo), (id_sb, ident)):
                nc.sync.dma_start(sb[:], dr[:])

            qt = qkpool.tile([f, seq], F16, tag="qt")
            kt = qkpool.tile([f, seq], F16, tag="kt")
            vsb = [vpool.tile([128, hl * dk1], F16, tag="v", name="v")
                   for _ in range(nst)]

            # ---- projections (feature-on-partition layout) ----
            with tc.tile_pool(name="vt", bufs=1) as vtpool:
                vt = vtpool.tile([f, seq], F16, tag="vt")
                with tc.tile_pool(name="xband", bufs=3) as xpool, \
                     tc.tile_pool(name="pproj", bufs=nsp,
                                  space="PSUM") as ppool:
                    for x_dr, w_sb, b_sb, dest in (
                            (xq, wq_sb, bq_sb, qt),
                            (xk, wk_sb, bk_sb, kt),
                            (xv, wv_sb, bv_sb, vt)):
                        psums = [ppool.tile([128, PS], F32, tag="pp", name="pp")
                                 for _ in range(nsp)]
                        for kc in range(kcn):
                            xb = xpool.tile([128, seq], F16, tag="xb")
                            nc.sync.dma_start(
                                xb[:], x_dr[kc * 128:(kc + 1) * 128, :])
                            for si in range(nsp):
                                nc.tensor.matmul(
                                    psums[si][:],
                                    lhsT=w_sb[:, kc * f:(kc + 1) * f],
                                    rhs=xb[:, si * PS:(si + 1) * PS],
                                    start=(kc == 0), stop=(kc == kcn - 1))
                        for si in range(nsp):
                            nc.vector.tensor_scalar_add(
                                dest[:, si * PS:(si + 1) * PS],
                                psums[si][:], b_sb[:])

                # ---- V -> natural [seq, feat] tiles with ones columns ----
                with tc.tile_pool(name="ptr", bufs=4, space="PSUM") as ptpool:
                    for st in range(nst):
                        pt = ptpool.tile([128, 128], F16, tag="pt")
                        nc.tensor.transpose(
                            pt[:], vt[:, st * 128:(st + 1) * 128], id_sb[:])
                        for hh in range(hl):
                            nc.vector.tensor_copy(
                                vsb[st][:, hh * dk1:hh * dk1 + dk],
                                pt[:, hh * dk:(hh + 1) * dk])
                            nc.vector.memset(
                                vsb[st][:, hh * dk1 + dk:hh * dk1 + dk1], 1.0)

            # ---- attention + output projection ----
            # Each (batch, head) is processed in query-windows of IW=1024
            # (2 PSUM strips): smaller pipeline quanta keep PE dense (3-deep
            # score buffering), shrink the per-window normalize/store burst,
            # and halve e-band residency.
            IW = 2 * PS if s % (2 * PS) == 0 else PS   # query window
            nwin = s // IW
            spw = IW // PS                             # strips per window
            with tc.tile_pool(name="eband", bufs=e_bufs) as epool, \
                 tc.tile_pool(name="psc", bufs=3, space="PSUM") as scpool, \
                 tc.tile_pool(name="pctx", bufs=spw, space="PSUM") as cxpool:
                for bb in range(b):
                    ctxb = ctxpool.tile([f, s], F16, tag="cb")
                    ctmp = ctxpool.tile([dk, s], F16, tag="ct", bufs=1,
                                        name="ctmp")
                    for hh in range(hl):
                        q_ap = qt[hh * dk:(hh + 1) * dk, bb * s:(bb + 1) * s]
                        k_ap = kt[hh * dk:(hh + 1) * dk, bb * s:(bb + 1) * s]
                        for iw in range(nwin):
                            i0 = iw * IW
                            ebands = [epool.tile([128, IW], F16, tag="eb",
                                                 name="eb")
                                      for _ in range(nj)]
                            cps = [cxpool.tile([dk1, PS], F32, tag="cx",
                                               name="cx")
                                   for _ in range(spw)]

                            def ctx_mm(jb):
                                vtile = vsb[bb * (s // 128) + jb]
                                for si in range(spw):
                                    nc.tensor.matmul(
                                        cps[si][:],
                                        lhsT=vtile[:, hh * dk1:
                                                   (hh + 1) * dk1],
                                        rhs=ebands[jb][:, si * PS:
                                                       (si + 1) * PS],
                                        start=(jb == 0), stop=(jb == nj - 1))

                            # scores^T + exp, pipelined one band ahead of
                            # the context matmuls so PE never waits on ACT
                            for jb in range(nj):
                                ps = scpool.tile([128, IW], F32,
                                                 tag="sc", name="sc")
                                for si in range(spw):
                                    nc.tensor.matmul(
                                        ps[:, si * PS:(si + 1) * PS],
                                        lhsT=k_ap[:, jb * 128:(jb + 1) * 128],
                                        rhs=q_ap[:, i0 + si * PS:
                                                 i0 + (si + 1) * PS],
                                        start=True, stop=True)
                                nc.scalar.activation(
                                    ebands[jb][:], ps[:], AF.Exp, scale=scale)
                                if jb > 0:
                                    ctx_mm(jb - 1)
                            ctx_mm(nj - 1)

                            # softmax denominators -> 1/sum. The row of sums
                            # (on partition dk) is DMA-reshaped to [IW/128,
                            # 128] so the exact DVE reciprocal runs on many
                            # lanes, then reshaped back and broadcast.
                            ivf = invpool.tile([dk1, IW], F32, tag="ivf",
                                               name="ivf", bufs=2)
                            for si in range(spw):
                                nc.vector.tensor_copy(
                                    ivf[dk:dk1, si * PS:(si + 1) * PS],
                                    cps[si][dk:dk1, :])
                            rs = invpool.tile([IW // 128, 128], F32,
                                              tag="rs", name="rs", bufs=2)
                            nc.sync.dma_start(rs[:], ivf[dk:dk1, :])
                            rsi = invpool.tile([IW // 128, 128], F16,
                                               tag="rsi", name="rsi", bufs=2)
                            with nc.allow_low_precision(
                                    reason="1/denom in fp16 matches the "
                                           "fp16 attn output rounding"):
                                nc.vector.reciprocal(rsi[:], rs[:])
                            inv16 = invpool.tile([1, IW], F16, tag="iv16",
                                                 bufs=2, name="iv16")
                            nc.sync.dma_start(inv16[:], rsi[:])
                            ivrep = invpool.tile([128, IW], F16, tag="ivp",
                                                 name="ivrep")
                            nc.gpsimd.partition_broadcast(ivrep[:], inv16[:])

                            # normalize context strips (h1 lands on
                            # partitions 0-63; DMA-shifted to 64-127 after
                            # both windows)
                            cdst = ctxb if hh == 0 else ctmp
                            for si in range(spw):
                                nc.vector.tensor_mul(
                                    cdst[0:dk, i0 + si * PS:
                                         i0 + (si + 1) * PS],
                                    cps[si][0:dk, :],
                                    ivrep[0:dk, si * PS:(si + 1) * PS])

                            # normalize e-bands in place and store attn^T
                            for jb in range(nj):
                                nc.vector.tensor_mul(
                                    ebands[jb][:], ebands[jb][:], ivrep[:])
                                nc.sync.dma_start(
                                    attn_o[bb, hh, jb * 128:(jb + 1) * 128,
                                           i0:i0 + IW],
                                    ebands[jb][:])
                        if hh == 1:
                            nc.sync.dma_start(ctxb[dk:f, :], ctmp[:])

                    # ---- output projection partial for batch bb ----
                    for et in range(net):
                        ob = opool.tile([128, s], F16, tag="ob")
                        for isx in range(ni):
                            ps = scpool.tile([128, PS], F32, tag="sc")
                            nc.tensor.matmul(
                                ps[:],
                                lhsT=wo_sb[:, et * 128:(et + 1) * 128],
                                rhs=ctxb[:, isx * PS:(isx + 1) * PS],
                                start=True, stop=True)
                            nc.vector.tensor_scalar_add(
                                ob[:, isx * PS:(isx + 1) * PS],
                                ps[:], bo_sb[:, et:et + 1])
                        nc.sync.dma_start(
                            out_o[et * 128:(et + 1) * 128,
                                  bb * s:(bb + 1) * s], ob[:])

    nc.compile()
    meta = dict(b=b, s=s, d=d, h=h, n_cores=n_cores, dk=dk, hl=hl, f=f,
                seq=seq, net=net)
    return nc, meta


def shard_inputs(inputs, meta):
    """Full fp32 inputs -> per-core in_maps (host-side layout prep only)."""
    b, s, d, h = meta["b"], meta["s"], meta["d"], meta["h"]
    n_cores, hl, f, net = meta["n_cores"], meta["hl"], meta["f"], meta["net"]
    kcn = d // 128
    seq = b * s

    def xt16(x):
        x16 = np.asarray(x, np.float32).reshape(seq, d).astype(np.float16)
        return np.ascontiguousarray(x16.T)

    XQ, XK, XV = (xt16(inputs[k]) for k in ("query", "key", "value"))
    ident = np.eye(128, dtype=np.float16)

    def wslice(W, fsel):
        # W[fsel].T as SBUF layout [128, kcn*f]
        wt = np.asarray(W, np.float32)[fsel, :].T.astype(np.float16)
        return np.ascontiguousarray(
            wt.reshape(kcn, 128, f).transpose(1, 0, 2).reshape(128, kcn * f))

    Wo = np.asarray(inputs["Wo"], np.float32)
    bo_tiled = np.ascontiguousarray(
        np.asarray(inputs["bo"], np.float32).reshape(net, 128).T)
    in_maps = []
    for c in range(n_cores):
        fsel = slice(c * f, (c + 1) * f)
        in_maps.append({
            "xq": XQ, "xk": XK, "xv": XV,
            "wq": wslice(inputs["Wq"], fsel),
            "wk": wslice(inputs["Wk"], fsel),
            "wv": wslice(inputs["Wv"], fsel),
            "wo": np.ascontiguousarray(
                Wo[:, fsel].T.astype(np.float16)),
            "bq": np.asarray(inputs["bq"], np.float32)[fsel].reshape(f, 1),
            "bk": np.asarray(inputs["bk"], np.float32)[fsel].reshape(f, 1),
            "bv": np.asarray(inputs["bv"], np.float32)[fsel].reshape(f, 1),
            "bo": bo_tiled if c == 0 else np.zeros((128, net), np.float32),
            "ident": ident,
        })
    return in_maps


def assemble_outputs(results, meta):
    b, s, d, h = meta["b"], meta["s"], meta["d"], meta["h"]
    n_cores, hl, seq = meta["n_cores"], meta["hl"], meta["seq"]
    attn = np.empty((b, h, s, s), np.float32)
    out_acc = np.zeros((d, seq), np.float32)
    for c in range(n_cores):
        a = results[c]["attn_o"]  # [b, hl, s(key), s(query)] fp16
        for bb in range(b):
            for hh in range(hl):
                attn[bb, c * hl + hh] = a[bb, hh].T.astype(np.float32)
        out_acc += results[c]["out_o"].astype(np.float32)
    out = np.ascontiguousarray(out_acc.T).reshape(b, s, d)
    return out, attn


_PROGRAM_CACHE = {}


def _get_program():
    if "nc" not in _PROGRAM_CACHE:
        nc, meta = build_program()
        _PROGRAM_CACHE["nc"] = nc
        _PROGRAM_CACHE["meta"] = meta
    return _PROGRAM_CACHE["nc"], _PROGRAM_CACHE["meta"]


def kernel(**inputs):
    nc, meta = _get_program()
    in_maps = shard_inputs(inputs, meta)
    res = bass_utils.run_bass_kernel_spmd(
        nc, in_maps, core_ids=list(range(meta["n_cores"])))
    return assemble_outputs(res.results, meta)


# revision 24
# speedup vs baseline: 1.3544x; 1.0040x over previous
"""Multi-head attention (16 heads, B=2, S=2048, D=1024) on 8 Trainium2
NeuronCores, tensor-parallel over heads (2 heads per core).

Contract: kernel(**inputs) takes the full unsharded fp32 inputs (as in the
reference nn.Module) and returns (output, attn) as full fp32 arrays.

Per-core program (identical code on all 8 cores; only input DATA differs):
  - inputs arrive pre-transposed/pre-cast on the host: x.T as [D, B*S] fp16,
    per-core weight slices in matmul-ready layouts.
  - Q/K/V projections for the core's 2 heads -> qT/kT [128, B*S] in SBUF
    (feature on partitions: head0 = partitions 0-63, head1 = 64-127).
    V is PE-transposed to natural [seq, feat] layout with an appended
    ones-column per head (fused row-sum trick).
  - Per (batch, local head): scores^T tiles [key=128, query=512] = K_h^T
    x Q_h strips on PE; exp via ACT (scale=1/sqrt(dk)) into fp16 e-bands;
    context^T = [V_h | 1]^T @ E accumulated in PSUM - row dk holds the
    softmax denominators for free.
  - Denominators -> reciprocal -> gpsimd partition-broadcast; DVE
    normalizes the e-bands in place (-> attn output, transposed layout)
    and the context strips.
  - Output projection partial = Wo_slice^T @ context^T -> [D, B*S] fp32
    per-core partial (bias bo fed only to core 0).
Host gathers: attn slabs are transposed back per head; partials are summed.
No cross-core collectives are needed.
"""

import numpy as np

import concourse.bacc as bacc
import concourse.mybir as mybir
import concourse.tile as tile
from concourse import bass_utils

# Problem dims (hardcoded per the harness contract)
B, S, D, H = 2, 2048, 1024, 16
N_CORES = 8

F16 = mybir.dt.float16
F32 = mybir.dt.float32
AF = mybir.ActivationFunctionType


def build_program(b=B, s=S, d=D, h=H, n_cores=N_CORES, e_bufs=44):
    """Build the (SPMD-identical) Bass program. Returns (nc, meta)."""
    dk = d // h                # head dim (64)
    hl = h // n_cores          # heads per core (2)
    f = hl * dk                # local projected features (128)
    seq = b * s                # total rows (4096)
    kcn = d // 128             # contraction chunks for projections (8)
    PS = 512                   # strip width (PSUM bank, fp32)
    nsp = seq // PS            # projection strips (8)
    ni = s // PS               # query strips per (b,h) (4)
    nj = s // 128              # key bands per (b,h) (16)
    nst = seq // 128           # V seq tiles (32)
    net = d // 128             # output-feature tiles (8)
    dk1 = dk + 1               # v cols + ones col
    assert f == 128, "per-core feature count must be 128"

    nc = bacc.Bacc("TRN2", target_bir_lowering=False, debug=False,
                   num_devices=n_cores)

    # ---- DRAM tensors (per-core views) ----
    xq = nc.dram_tensor("xq", [d, seq], F16, kind="ExternalInput").ap()
    xk = nc.dram_tensor("xk", [d, seq], F16, kind="ExternalInput").ap()
    xv = nc.dram_tensor("xv", [d, seq], F16, kind="ExternalInput").ap()
    wq = nc.dram_tensor("wq", [128, kcn * f], F16, kind="ExternalInput").ap()
    wk = nc.dram_tensor("wk", [128, kcn * f], F16, kind="ExternalInput").ap()
    wv = nc.dram_tensor("wv", [128, kcn * f], F16, kind="ExternalInput").ap()
    wo = nc.dram_tensor("wo", [f, d], F16, kind="ExternalInput").ap()
    bq = nc.dram_tensor("bq", [f, 1], F32, kind="ExternalInput").ap()
    bk = nc.dram_tensor("bk", [f, 1], F32, kind="ExternalInput").ap()
    bv = nc.dram_tensor("bv", [f, 1], F32, kind="ExternalInput").ap()
    ident = nc.dram_tensor("ident", [128, 128], F16, kind="ExternalInput").ap()

    attn_o = nc.dram_tensor("attn_o", [b, hl, s, s], F16,
                            kind="ExternalOutput").ap()
    out_o = nc.dram_tensor("out_o", [d, seq], F16,
                           kind="ExternalOutput").ap()

    scale = 1.0 / float(np.sqrt(dk))

    with tile.TileContext(nc) as tc:
        with tc.tile_pool(name="const", bufs=1) as cpool, \
             tc.tile_pool(name="qk", bufs=1) as qkpool, \
             tc.tile_pool(name="vsb", bufs=nst) as vpool, \
             tc.tile_pool(name="ctxp", bufs=2) as ctxpool, \
             tc.tile_pool(name="invp", bufs=2) as invpool, \
             tc.tile_pool(name="obufp", bufs=2) as opool:

            # ---- constants ----
            wq_sb = cpool.tile([128, kcn * f], F16, tag="wq")
            wk_sb = cpool.tile([128, kcn * f], F16, tag="wk")
            wv_sb = cpool.tile([128, kcn * f], F16, tag="wv")
            wo_sb = cpool.tile([f, d], F16, tag="wo")
            bq_sb = cpool.tile([f, 1], F32, tag="bq")
            bk_sb = cpool.tile([f, 1], F32, tag="bk")
            bv_sb = cpool.tile([f, 1], F32, tag="bv")
            id_sb = cpool.tile([128, 128], F16, tag="id")
            for sb, dr in ((wq_sb, wq), (wk_sb, wk), (wv_sb, wv),
                           (wo_sb, wo), (bq_sb, bq), (bk_sb, bk),
                           (bv_sb, bv), (id_sb, ident)):
                nc.sync.dma_start(sb[:], dr[:])

            qt = qkpool.tile([f, seq], F16, tag="qt")
            kt = qkpool.tile([f, seq], F16, tag="kt")
            vsb = [vpool.tile([128, hl * dk1], F16, tag="v", name="v")
                   for _ in range(nst)]

            # ---- projections (feature-on-partition layout) ----
            with tc.tile_pool(name="vt", bufs=1) as vtpool:
                vt = vtpool.tile([f, seq], F16, tag="vt")
                with tc.tile_pool(name="xband", bufs=3) as xpool, \
                     tc.tile_pool(name="pproj", bufs=nsp,
                                  space="PSUM") as ppool:
                    for x_dr, w_sb, b_sb, dest in (
                            (xq, wq_sb, bq_sb, qt),
                            (xk, wk_sb, bk_sb, kt),
                            (xv, wv_sb, bv_sb, vt)):
                        psums = [ppool.tile([128, PS], F32, tag="pp", name="pp")
                                 for _ in range(nsp)]
                        for kc in range(kcn):
                            xb = xpool.tile([128, seq], F16, tag="xb")
                            nc.sync.dma_start(
                                xb[:], x_dr[kc * 128:(kc + 1) * 128, :])
                            for si in range(nsp):
                                nc.tensor.matmul(
                                    psums[si][:],
                                    lhsT=w_sb[:, kc * f:(kc + 1) * f],
                                    rhs=xb[:, si * PS:(si + 1) * PS],
                                    start=(kc == 0), stop=(kc == kcn - 1))
                        for si in range(nsp):
                            nc.vector.tensor_scalar_add(
                                dest[:, si * PS:(si + 1) * PS],
                                psums[si][:], b_sb[:])

                # ---- V -> natural [seq, feat] tiles with ones columns ----
                with tc.tile_pool(name="ptr", bufs=4, space="PSUM") as ptpool:
                    for st in range(nst):
                        pt = ptpool.tile([128, 128], F16, tag="pt")
                        nc.tensor.transpose(
                            pt[:], vt[:, st * 128:(st + 1) * 128], id_sb[:])
                        for hh in range(hl):
                            nc.vector.tensor_copy(
                                vsb[st][:, hh * dk1:hh * dk1 + dk],
                                pt[:, hh * dk:(hh + 1) * dk])
                            nc.vector.memset(
                                vsb[st][:, hh * dk1 + dk:hh * dk1 + dk1], 1.0)

            # ---- attention + output projection ----
            # Each (batch, head) is processed in query-windows of IW=1024
            # (2 PSUM strips): smaller pipeline quanta keep PE dense (3-deep
            # score buffering), shrink the per-window normalize/store burst,
            # and halve e-band residency.
            IW = 2 * PS if s % (2 * PS) == 0 else PS   # query window
            nwin = s // IW
            spw = IW // PS                             # strips per window
            with tc.tile_pool(name="eband", bufs=e_bufs) as epool, \
                 tc.tile_pool(name="psc", bufs=3, space="PSUM") as scpool, \
                 tc.tile_pool(name="pctx", bufs=spw, space="PSUM") as cxpool:

                def emit_outproj(bb, ctxb):
                    # partial output projection for batch bb; PSUM->SBUF
                    # copies run on ACT (plain Copy, no table) so DVE keeps
                    # pace freeing e-band slots. bo is folded into the
                    # host-side partial-sum reduction.
                    for et in range(net):
                        ob = opool.tile([128, s], F16, tag="ob", name="ob")
                        for isx in range(ni):
                            ps = scpool.tile([128, PS], F32, tag="sc",
                                             name="sc")
                            nc.tensor.matmul(
                                ps[:],
                                lhsT=wo_sb[:, et * 128:(et + 1) * 128],
                                rhs=ctxb[:, isx * PS:(isx + 1) * PS],
                                start=True, stop=True)
                            nc.scalar.activation(
                                ob[:, isx * PS:(isx + 1) * PS],
                                ps[:], AF.Copy)
                        nc.sync.dma_start(
                            out_o[et * 128:(et + 1) * 128,
                                  bb * s:(bb + 1) * s], ob[:])

                ctxbs = {}
                for bb in range(b):
                    ctxb = ctxpool.tile([f, s], F16, tag="cb")
                    ctxbs[bb] = ctxb
                    ctmp = ctxpool.tile([dk, s], F16, tag="ct", bufs=1,
                                        name="ctmp")
                    for hh in range(hl):
                        q_ap = qt[hh * dk:(hh + 1) * dk, bb * s:(bb + 1) * s]
                        k_ap = kt[hh * dk:(hh + 1) * dk, bb * s:(bb + 1) * s]
                        for iw in range(nwin):
                            i0 = iw * IW
                            ebands = [epool.tile([128, IW], F16, tag="eb",
                                                 name="eb")
                                      for _ in range(nj)]
                            cps = [cxpool.tile([dk1, PS], F32, tag="cx",
                                               name="cx")
                                   for _ in range(spw)]

                            def ctx_mm(jb):
                                vtile = vsb[bb * (s // 128) + jb]
                                for si in range(spw):
                                    nc.tensor.matmul(
                                        cps[si][:],
                                        lhsT=vtile[:, hh * dk1:
                                                   (hh + 1) * dk1],
                                        rhs=ebands[jb][:, si * PS:
                                                       (si + 1) * PS],
                                        start=(jb == 0), stop=(jb == nj - 1))

                            # scores^T + exp, pipelined one band ahead of
                            # the context matmuls so PE never waits on ACT
                            for jb in range(nj):
                                ps = scpool.tile([128, IW], F32,
                                                 tag="sc", name="sc")
                                for si in range(spw):
                                    nc.tensor.matmul(
                                        ps[:, si * PS:(si + 1) * PS],
                                        lhsT=k_ap[:, jb * 128:(jb + 1) * 128],
                                        rhs=q_ap[:, i0 + si * PS:
                                                 i0 + (si + 1) * PS],
                                        start=True, stop=True)
                                nc.scalar.activation(
                                    ebands[jb][:], ps[:], AF.Exp, scale=scale)
                                if jb > 0:
                                    ctx_mm(jb - 1)
                            ctx_mm(nj - 1)

                            # softmax denominators -> 1/sum. The row of sums
                            # (on partition dk) is DMA-reshaped to [IW/128,
                            # 128] so the exact DVE reciprocal runs on many
                            # lanes, then reshaped back and broadcast.
                            ivf = invpool.tile([dk1, IW], F32, tag="ivf",
                                               name="ivf", bufs=2)
                            for si in range(spw):
                                nc.vector.tensor_copy(
                                    ivf[dk:dk1, si * PS:(si + 1) * PS],
                                    cps[si][dk:dk1, :])
                            rs = invpool.tile([IW // 128, 128], F32,
                                              tag="rs", name="rs", bufs=2)
                            nc.sync.dma_start(rs[:], ivf[dk:dk1, :])
                            rsi = invpool.tile([IW // 128, 128], F16,
                                               tag="rsi", name="rsi", bufs=2)
                            with nc.allow_low_precision(
                                    reason="1/denom in fp16 matches the "
                                           "fp16 attn output rounding"):
                                nc.vector.reciprocal(rsi[:], rs[:])
                            inv16 = invpool.tile([1, IW], F16, tag="iv16",
                                                 bufs=2, name="iv16")
                            nc.sync.dma_start(inv16[:], rsi[:])
                            ivrep = invpool.tile([128, IW], F16, tag="ivp",
                                                 name="ivrep")
                            nc.gpsimd.partition_broadcast(ivrep[:], inv16[:])

                            # normalize context strips (h1 lands on
                            # partitions 0-63; DMA-shifted to 64-127 after
                            # both windows)
                            cdst = ctxb if hh == 0 else ctmp
                            for si in range(spw):
                                nc.vector.tensor_mul(
                                    cdst[0:dk, i0 + si * PS:
                                         i0 + (si + 1) * PS],
                                    cps[si][0:dk, :],
                                    ivrep[0:dk, si * PS:(si + 1) * PS])

                            # normalize e-bands in place and store attn^T
                            for jb in range(nj):
                                nc.vector.tensor_mul(
                                    ebands[jb][:], ebands[jb][:], ivrep[:])
                                nc.sync.dma_start(
                                    attn_o[bb, hh, jb * 128:(jb + 1) * 128,
                                           i0:i0 + IW],
                                    ebands[jb][:])
                            # emit the previous batch's output projection
                            # here so its matmuls fill PE while this
                            # window's softmax/normalize chain drains
                            if bb > 0 and hh == 0 and iw == 0:
                                emit_outproj(bb - 1, ctxbs[bb - 1])
                        if hh == 1:
                            nc.sync.dma_start(ctxb[dk:f, :], ctmp[:])

                emit_outproj(b - 1, ctxbs[b - 1])

    nc.compile()
    meta = dict(b=b, s=s, d=d, h=h, n_cores=n_cores, dk=dk, hl=hl, f=f,
                seq=seq, net=net)
    return nc, meta


def shard_inputs(inputs, meta):
    """Full fp32 inputs -> per-core in_maps (host-side layout prep only)."""
    b, s, d, h = meta["b"], meta["s"], meta["d"], meta["h"]
    n_cores, hl, f, net = meta["n_cores"], meta["hl"], meta["f"], meta["net"]
    kcn = d // 128
    seq = b * s

    def xt16(x):
        x16 = np.asarray(x, np.float32).reshape(seq, d).astype(np.float16)
        return np.ascontiguousarray(x16.T)

    XQ, XK, XV = (xt16(inputs[k]) for k in ("query", "key", "value"))
    ident = np.eye(128, dtype=np.float16)

    def wslice(W, fsel):
        # W[fsel].T as SBUF layout [128, kcn*f]
        wt = np.asarray(W, np.float32)[fsel, :].T.astype(np.float16)
        return np.ascontiguousarray(
            wt.reshape(kcn, 128, f).transpose(1, 0, 2).reshape(128, kcn * f))

    Wo = np.asarray(inputs["Wo"], np.float32)
    in_maps = []
    for c in range(n_cores):
        fsel = slice(c * f, (c + 1) * f)
        in_maps.append({
            "xq": XQ, "xk": XK, "xv": XV,
            "wq": wslice(inputs["Wq"], fsel),
            "wk": wslice(inputs["Wk"], fsel),
            "wv": wslice(inputs["Wv"], fsel),
            "wo": np.ascontiguousarray(
                Wo[:, fsel].T.astype(np.float16)),
            "bq": np.asarray(inputs["bq"], np.float32)[fsel].reshape(f, 1),
            "bk": np.asarray(inputs["bk"], np.float32)[fsel].reshape(f, 1),
            "bv": np.asarray(inputs["bv"], np.float32)[fsel].reshape(f, 1),
            "ident": ident,
        })
    return in_maps


def assemble_outputs(results, meta, bo):
    b, s, d, h = meta["b"], meta["s"], meta["d"], meta["h"]
    n_cores, hl, seq = meta["n_cores"], meta["hl"], meta["seq"]
    attn = np.empty((b, h, s, s), np.float32)
    out_acc = np.zeros((d, seq), np.float32)
    for c in range(n_cores):
        a = results[c]["attn_o"]  # [b, hl, s(key), s(query)] fp16
        for bb in range(b):
            for hh in range(hl):
                attn[bb, c * hl + hh] = a[bb, hh].T.astype(np.float32)
        out_acc += results[c]["out_o"].astype(np.float32)
    out = np.ascontiguousarray(out_acc.T)
    out += np.asarray(bo, np.float32)[None, :]
    return out.reshape(b, s, d), attn


_PROGRAM_CACHE = {}


def _get_program():
    if "nc" not in _PROGRAM_CACHE:
        nc, meta = build_program()
        _PROGRAM_CACHE["nc"] = nc
        _PROGRAM_CACHE["meta"] = meta
    return _PROGRAM_CACHE["nc"], _PROGRAM_CACHE["meta"]


def kernel(**inputs):
    nc, meta = _get_program()
    in_maps = shard_inputs(inputs, meta)
    res = bass_utils.run_bass_kernel_spmd(
        nc, in_maps, core_ids=list(range(meta["n_cores"])))
    return assemble_outputs(res.results, meta, inputs["bo"])


# revision 25
# speedup vs baseline: 1.4985x; 1.1064x over previous
"""Multi-head attention (16 heads, B=2, S=2048, D=1024) on 8 Trainium2
NeuronCores, tensor-parallel over heads (2 heads per core).

Contract: kernel(**inputs) takes the full unsharded fp32 inputs (as in the
reference nn.Module) and returns (output, attn) as full fp32 arrays.

Per-core program (identical code on all 8 cores; only input DATA differs):
  - inputs arrive pre-transposed/pre-cast on the host: x.T as [D, B*S] fp16,
    per-core weight slices in matmul-ready layouts.
  - Q/K/V projections for the core's 2 heads -> qT/kT [128, B*S] in SBUF
    (feature on partitions: head0 = partitions 0-63, head1 = 64-127).
    V is PE-transposed to natural [seq, feat] layout with an appended
    ones-column per head (fused row-sum trick).
  - Per (batch, local head): scores^T tiles [key=128, query=512] = K_h^T
    x Q_h strips on PE; exp via ACT (scale=1/sqrt(dk)) into fp16 e-bands;
    context^T = [V_h | 1]^T @ E accumulated in PSUM - row dk holds the
    softmax denominators for free.
  - Denominators -> reciprocal -> gpsimd partition-broadcast; DVE
    normalizes the e-bands in place (-> attn output, transposed layout)
    and the context strips.
  - Output projection partial = Wo_slice^T @ context^T -> [D, B*S] fp32
    per-core partial (bias bo fed only to core 0).
Host gathers: attn slabs are transposed back per head; partials are summed.
No cross-core collectives are needed.
"""

import numpy as np

import concourse.bacc as bacc
import concourse.mybir as mybir
import concourse.tile as tile
from concourse import bass_utils

# Problem dims (hardcoded per the harness contract)
B, S, D, H = 2, 2048, 1024, 16
N_CORES = 8

F16 = mybir.dt.float16
F32 = mybir.dt.float32
AF = mybir.ActivationFunctionType


def build_program(b=B, s=S, d=D, h=H, n_cores=N_CORES, e_bufs=44):
    """Build the (SPMD-identical) Bass program. Returns (nc, meta)."""
    dk = d // h                # head dim (64)
    hl = h // n_cores          # heads per core (2)
    f = hl * dk                # local projected features (128)
    seq = b * s                # total rows (4096)
    kcn = d // 128             # contraction chunks for projections (8)
    PS = 512                   # strip width (PSUM bank, fp32)
    nsp = seq // PS            # projection strips (8)
    ni = s // PS               # query strips per (b,h) (4)
    nj = s // 128              # key bands per (b,h) (16)
    nst = seq // 128           # V seq tiles (32)
    net = d // 128             # output-feature tiles (8)
    dk1 = dk + 1               # v cols + ones col
    assert f == 128, "per-core feature count must be 128"

    nc = bacc.Bacc("TRN2", target_bir_lowering=False, debug=False,
                   num_devices=n_cores)

    # ---- DRAM tensors (per-core views) ----
    xq = nc.dram_tensor("xq", [d, seq], F16, kind="ExternalInput").ap()
    xk = nc.dram_tensor("xk", [d, seq], F16, kind="ExternalInput").ap()
    xv = nc.dram_tensor("xv", [d, seq], F16, kind="ExternalInput").ap()
    wq = nc.dram_tensor("wq", [128, kcn * f], F16, kind="ExternalInput").ap()
    wk = nc.dram_tensor("wk", [128, kcn * f], F16, kind="ExternalInput").ap()
    wv = nc.dram_tensor("wv", [128, kcn * f], F16, kind="ExternalInput").ap()
    wo = nc.dram_tensor("wo", [f, d], F16, kind="ExternalInput").ap()
    bq = nc.dram_tensor("bq", [f, 1], F32, kind="ExternalInput").ap()
    bk = nc.dram_tensor("bk", [f, 1], F32, kind="ExternalInput").ap()
    bv = nc.dram_tensor("bv", [f, 1], F32, kind="ExternalInput").ap()
    ident = nc.dram_tensor("ident", [128, 128], F16, kind="ExternalInput").ap()

    attn_o = nc.dram_tensor("attn_o", [b, hl, s, s], F16,
                            kind="ExternalOutput").ap()
    out_o = nc.dram_tensor("out_o", [d, seq], F16,
                           kind="ExternalOutput").ap()

    scale = 1.0 / float(np.sqrt(dk))

    with tile.TileContext(nc) as tc:
        with tc.tile_pool(name="const", bufs=1) as cpool, \
             tc.tile_pool(name="qk", bufs=1) as qkpool, \
             tc.tile_pool(name="vsb", bufs=nst) as vpool, \
             tc.tile_pool(name="ctxp", bufs=2) as ctxpool, \
             tc.tile_pool(name="invp", bufs=2) as invpool, \
             tc.tile_pool(name="obufp", bufs=4) as opool:

            # ---- constants ----
            wq_sb = cpool.tile([128, kcn * f], F16, tag="wq")
            wk_sb = cpool.tile([128, kcn * f], F16, tag="wk")
            wv_sb = cpool.tile([128, kcn * f], F16, tag="wv")
            wo_sb = cpool.tile([f, d], F16, tag="wo")
            bq_sb = cpool.tile([f, 1], F32, tag="bq")
            bk_sb = cpool.tile([f, 1], F32, tag="bk")
            bv_sb = cpool.tile([f, 1], F32, tag="bv")
            id_sb = cpool.tile([128, 128], F16, tag="id")
            for sb, dr in ((wq_sb, wq), (wk_sb, wk), (wv_sb, wv),
                           (wo_sb, wo), (bq_sb, bq), (bk_sb, bk),
                           (bv_sb, bv), (id_sb, ident)):
                nc.sync.dma_start(sb[:], dr[:])

            qt = qkpool.tile([f, seq], F16, tag="qt")
            kt = qkpool.tile([f, seq], F16, tag="kt")
            vsb = [vpool.tile([128, hl * dk1], F16, tag="v", name="v")
                   for _ in range(nst)]

            # ---- projections (feature-on-partition layout) ----
            with tc.tile_pool(name="vt", bufs=1) as vtpool:
                vt = vtpool.tile([f, seq], F16, tag="vt")
                with tc.tile_pool(name="xband", bufs=4) as xpool, \
                     tc.tile_pool(name="pproj", bufs=nsp,
                                  space="PSUM") as ppool:
                    for x_dr, w_sb, b_sb, dest in (
                            (xq, wq_sb, bq_sb, qt),
                            (xk, wk_sb, bk_sb, kt),
                            (xv, wv_sb, bv_sb, vt)):
                        psums = [ppool.tile([128, PS], F32, tag="pp", name="pp")
                                 for _ in range(nsp)]
                        for kc in range(kcn):
                            xb = xpool.tile([128, seq], F16, tag="xb")
                            nc.sync.dma_start(
                                xb[:], x_dr[kc * 128:(kc + 1) * 128, :])
                            for si in range(nsp):
                                nc.tensor.matmul(
                                    psums[si][:],
                                    lhsT=w_sb[:, kc * f:(kc + 1) * f],
                                    rhs=xb[:, si * PS:(si + 1) * PS],
                                    start=(kc == 0), stop=(kc == kcn - 1))
                        for si in range(nsp):
                            nc.vector.tensor_scalar_add(
                                dest[:, si * PS:(si + 1) * PS],
                                psums[si][:], b_sb[:])

                # ---- V -> natural [seq, feat] tiles with ones columns ----
                with tc.tile_pool(name="ptr", bufs=4, space="PSUM") as ptpool:
                    for st in range(nst):
                        pt = ptpool.tile([128, 128], F16, tag="pt")
                        nc.tensor.transpose(
                            pt[:], vt[:, st * 128:(st + 1) * 128], id_sb[:])
                        for hh in range(hl):
                            nc.vector.tensor_copy(
                                vsb[st][:, hh * dk1:hh * dk1 + dk],
                                pt[:, hh * dk:(hh + 1) * dk])
                            nc.vector.memset(
                                vsb[st][:, hh * dk1 + dk:hh * dk1 + dk1], 1.0)

            # ---- attention + output projection ----
            # Each (batch, head) is processed in query-windows of IW=1024
            # (2 PSUM strips): smaller pipeline quanta keep PE dense (3-deep
            # score buffering), shrink the per-window normalize/store burst,
            # and halve e-band residency.
            IW = 2 * PS if s % (2 * PS) == 0 else PS   # query window
            nwin = s // IW
            spw = IW // PS                             # strips per window
            with tc.tile_pool(name="eband", bufs=e_bufs) as epool, \
                 tc.tile_pool(name="psc", bufs=3, space="PSUM") as scpool, \
                 tc.tile_pool(name="pctx", bufs=spw, space="PSUM") as cxpool:

                def emit_outproj(bb, ctxb):
                    # partial output projection for batch bb; PSUM->SBUF
                    # copies run on ACT (plain Copy, no table) so DVE keeps
                    # pace freeing e-band slots. bo is folded into the
                    # host-side partial-sum reduction.
                    for et in range(net):
                        ob = opool.tile([128, s], F16, tag="ob", name="ob")
                        for isx in range(ni):
                            ps = scpool.tile([128, PS], F32, tag="sc",
                                             name="sc")
                            nc.tensor.matmul(
                                ps[:],
                                lhsT=wo_sb[:, et * 128:(et + 1) * 128],
                                rhs=ctxb[:, isx * PS:(isx + 1) * PS],
                                start=True, stop=True)
                            nc.scalar.activation(
                                ob[:, isx * PS:(isx + 1) * PS],
                                ps[:], AF.Copy)
                        nc.sync.dma_start(
                            out_o[et * 128:(et + 1) * 128,
                                  bb * s:(bb + 1) * s], ob[:])

                ctxbs = {}
                for bb in range(b):
                    ctxb = ctxpool.tile([f, s], F16, tag="cb")
                    ctxbs[bb] = ctxb
                    ctmp = ctxpool.tile([dk, s], F16, tag="ct", bufs=1,
                                        name="ctmp")
                    for hh in range(hl):
                        q_ap = qt[hh * dk:(hh + 1) * dk, bb * s:(bb + 1) * s]
                        k_ap = kt[hh * dk:(hh + 1) * dk, bb * s:(bb + 1) * s]
                        for iw in range(nwin):
                            i0 = iw * IW
                            ebands = [epool.tile([128, IW], F16, tag="eb",
                                                 name="eb")
                                      for _ in range(nj)]
                            cps = [cxpool.tile([dk1, PS], F32, tag="cx",
                                               name="cx")
                                   for _ in range(spw)]

                            def ctx_mm(jb):
                                vtile = vsb[bb * (s // 128) + jb]
                                for si in range(spw):
                                    nc.tensor.matmul(
                                        cps[si][:],
                                        lhsT=vtile[:, hh * dk1:
                                                   (hh + 1) * dk1],
                                        rhs=ebands[jb][:, si * PS:
                                                       (si + 1) * PS],
                                        start=(jb == 0), stop=(jb == nj - 1))

                            # scores^T + exp, pipelined one band ahead of
                            # the context matmuls so PE never waits on ACT
                            for jb in range(nj):
                                ps = scpool.tile([128, IW], F32,
                                                 tag="sc", name="sc")
                                for si in range(spw):
                                    nc.tensor.matmul(
                                        ps[:, si * PS:(si + 1) * PS],
                                        lhsT=k_ap[:, jb * 128:(jb + 1) * 128],
                                        rhs=q_ap[:, i0 + si * PS:
                                                 i0 + (si + 1) * PS],
                                        start=True, stop=True)
                                nc.scalar.activation(
                                    ebands[jb][:], ps[:], AF.Exp, scale=scale)
                                if jb > 0:
                                    ctx_mm(jb - 1)
                            ctx_mm(nj - 1)

                            # softmax denominators -> 1/sum. The row of sums
                            # (on partition dk) is DMA-reshaped to [IW/128,
                            # 128] so the exact DVE reciprocal runs on many
                            # lanes, then reshaped back and broadcast.
                            ivf = invpool.tile([dk1, IW], F32, tag="ivf",
                                               name="ivf", bufs=2)
                            for si in range(spw):
                                nc.vector.tensor_copy(
                                    ivf[dk:dk1, si * PS:(si + 1) * PS],
                                    cps[si][dk:dk1, :])
                            rs = invpool.tile([IW // 128, 128], F32,
                                              tag="rs", name="rs", bufs=2)
                            nc.gpsimd.dma_start(rs[:], ivf[dk:dk1, :])
                            rsi = invpool.tile([IW // 128, 128], F16,
                                               tag="rsi", name="rsi", bufs=2)
                            with nc.allow_low_precision(
                                    reason="1/denom in fp16 matches the "
                                           "fp16 attn output rounding"):
                                nc.vector.reciprocal(rsi[:], rs[:])
                            inv16 = invpool.tile([1, IW], F16, tag="iv16",
                                                 bufs=2, name="iv16")
                            nc.gpsimd.dma_start(inv16[:], rsi[:])
                            ivrep = invpool.tile([128, IW], F16, tag="ivp",
                                                 name="ivrep")
                            nc.gpsimd.partition_broadcast(ivrep[:], inv16[:])

                            # normalize context strips (h1 lands on
                            # partitions 0-63; DMA-shifted to 64-127 after
                            # both windows)
                            cdst = ctxb if hh == 0 else ctmp
                            for si in range(spw):
                                nc.vector.tensor_mul(
                                    cdst[0:dk, i0 + si * PS:
                                         i0 + (si + 1) * PS],
                                    cps[si][0:dk, :],
                                    ivrep[0:dk, si * PS:(si + 1) * PS])

                            # normalize e-bands in place and store attn^T
                            for jb in range(nj):
                                nc.vector.tensor_mul(
                                    ebands[jb][:], ebands[jb][:], ivrep[:])
                                nc.sync.dma_start(
                                    attn_o[bb, hh, jb * 128:(jb + 1) * 128,
                                           i0:i0 + IW],
                                    ebands[jb][:])
                            # emit the previous batch's output projection
                            # here so its matmuls fill PE while this
                            # window's softmax/normalize chain drains
                            if bb > 0 and hh == 0 and iw == 0:
                                emit_outproj(bb - 1, ctxbs[bb - 1])
                        if hh == 1:
                            nc.gpsimd.dma_start(ctxb[dk:f, :], ctmp[:])

                emit_outproj(b - 1, ctxbs[b - 1])

    nc.compile()
    meta = dict(b=b, s=s, d=d, h=h, n_cores=n_cores, dk=dk, hl=hl, f=f,
                seq=seq, net=net)
    return nc, meta


def shard_inputs(inputs, meta):
    """Full fp32 inputs -> per-core in_maps (host-side layout prep only)."""
    b, s, d, h = meta["b"], meta["s"], meta["d"], meta["h"]
    n_cores, hl, f, net = meta["n_cores"], meta["hl"], meta["f"], meta["net"]
    kcn = d // 128
    seq = b * s

    def xt16(x):
        x16 = np.asarray(x, np.float32).reshape(seq, d).astype(np.float16)
        return np.ascontiguousarray(x16.T)

    XQ, XK, XV = (xt16(inputs[k]) for k in ("query", "key", "value"))
    ident = np.eye(128, dtype=np.float16)

    def wslice(W, fsel):
        # W[fsel].T as SBUF layout [128, kcn*f]
        wt = np.asarray(W, np.float32)[fsel, :].T.astype(np.float16)
        return np.ascontiguousarray(
            wt.reshape(kcn, 128, f).transpose(1, 0, 2).reshape(128, kcn * f))

    Wo = np.asarray(inputs["Wo"], np.float32)
    in_maps = []
    for c in range(n_cores):
        fsel = slice(c * f, (c + 1) * f)
        in_maps.append({
            "xq": XQ, "xk": XK, "xv": XV,
            "wq": wslice(inputs["Wq"], fsel),
            "wk": wslice(inputs["Wk"], fsel),
            "wv": wslice(inputs["Wv"], fsel),
            "wo": np.ascontiguousarray(
                Wo[:, fsel].T.astype(np.float16)),
            "bq": np.asarray(inputs["bq"], np.float32)[fsel].reshape(f, 1),
            "bk": np.asarray(inputs["bk"], np.float32)[fsel].reshape(f, 1),
            "bv": np.asarray(inputs["bv"], np.float32)[fsel].reshape(f, 1),
            "ident": ident,
        })
    return in_maps


def assemble_outputs(results, meta, bo):
    b, s, d, h = meta["b"], meta["s"], meta["d"], meta["h"]
    n_cores, hl, seq = meta["n_cores"], meta["hl"], meta["seq"]
    attn = np.empty((b, h, s, s), np.float32)
    out_acc = np.zeros((d, seq), np.float32)
    for c in range(n_cores):
        a = results[c]["attn_o"]  # [b, hl, s(key), s(query)] fp16
        for bb in range(b):
            for hh in range(hl):
                attn[bb, c * hl + hh] = a[bb, hh].T.astype(np.float32)
        out_acc += results[c]["out_o"].astype(np.float32)
    out = np.ascontiguousarray(out_acc.T)
    out += np.asarray(bo, np.float32)[None, :]
    return out.reshape(b, s, d), attn


_PROGRAM_CACHE = {}


def _get_program():
    if "nc" not in _PROGRAM_CACHE:
        nc, meta = build_program()
        _PROGRAM_CACHE["nc"] = nc
        _PROGRAM_CACHE["meta"] = meta
    return _PROGRAM_CACHE["nc"], _PROGRAM_CACHE["meta"]


def kernel(**inputs):
    nc, meta = _get_program()
    in_maps = shard_inputs(inputs, meta)
    res = bass_utils.run_bass_kernel_spmd(
        nc, in_maps, core_ids=list(range(meta["n_cores"])))
    return assemble_outputs(res.results, meta, inputs["bo"])
